# revision 16
# baseline (speedup 1.0000x reference)
"""Trainium2 Bass kernel for a 3x3 stride-1 pad-1 Conv2d.

Problem: x (16, 64, 112, 112) f32, weights (128, 64, 9) f32
         -> out (16, 128, 112, 112) f32  (no bias)

Strategy (8 NeuronCores, data parallel over batch):
  - Each core gets 2 images. Image 0 lives in SBUF partitions 0-63
    (64 input channels), image 1 in partitions 64-127, both stored as a
    zero-padded (114, 114) plane per channel. Padding is materialized on
    the host, so every input DMA is a contiguous fat-descriptor copy.
  - Everything is bf16 end-to-end (inputs, weights, staged outputs);
    PSUM accumulation stays fp32. bf16 halves HBM traffic and enables
    the PE's fast-weight-load path (FWL reads 2 bf16/cycle), which
    matters because LDWEIGHTS (128 cols @ 1.2 GHz) is otherwise ~45% of
    the PE-stream critical path. The host quantizes x/w to bf16 and
    upcasts the output; total rel-err ~3e-3 (gate is 2e-2).
  - Conv = 9 shift-and-matmul taps accumulated in PSUM: for each tap
    (dy, dx), matmul with lhsT = w[tap] (64 x 128: in-ch x out-ch) and
    rhs = shifted x window (64 x 448: in-ch x 4 output rows).
  - The two images' matmuls use disjoint PE row groups (rows 0-63 vs
    64-127 via tile_position) so they stream concurrently -> together
    they fill the whole 128x128 array despite the 64-deep contraction.
    Steady-state pairs run at the 448-cycle floor (~190 ns/pair).
  - Warm-up matmuls on zeroed scratch run during the DMA head so the
    PE_HAM clock gate un-throttles (1.2 -> 2.4 GHz) ASAP; the burst is
    sized to end right as the first input band lands (ending early
    risks an idle gap that slips the un-throttle by a whole HAM window).
  - Input bands are completion-chained at depth 2 (band b waits on band
    b-2) so the head band + weights get the SDMA rings to themselves --
    the rings round-robin across ACTIVE queues, so unchained later
    bands would steal head bandwidth. Bands 2+ are issued from GpSimd.
  - PSUM -> SBUF copies (f32->bf16 cast) run on ScalarE (image 0) and
    VectorE (image 1); stores are issued from ScalarE (im 0) and Sync
    (im 1) per 16-row band. The final block is split into two 2-row
    PSUM groups so its copy+store tail is halved.
"""

import numpy as np
import ml_dtypes

import concourse.bass as bass
import concourse.bacc as bacc
import concourse.mybir as mybir
import concourse.tile as tile
from concourse.bass_utils import run_bass_kernel_spmd
from concourse.tile_rust import add_dep_helper

N_CORES = 8
B, C, H, W = 16, 64, 112, 112
O = 128
BPC = B // N_CORES          # images per core
HP = H + 2                  # padded rows per image plane
WP = W + 2                  # padded cols
NTAPS = 9
RPB = 4                     # output rows per block (free dim = 4*112 = 448)
NBLOCKS = H // RPB          # 28
BAND = 16                   # output rows per output band
NBANDS = H // BAND          # 7
NWARM = 16                  # PE warm-up matmuls (8 quadrant pairs)

F32 = mybir.dt.float32
BF16 = mybir.dt.bfloat16
BF16NP = ml_dtypes.bfloat16

# input bands over padded rows: (first padded row, nrows). The head band
# covers block 0; band b>=2 is completion-chained on band b-2.
_IN_BANDS = [(0, 6), (6, 16), (22, 16), (38, 16), (54, 16), (70, 16),
             (86, 16), (102, 12)]


def _conv_body(tc, out_ap, xp_ap, w_ap):
    nc = tc.nc
    from contextlib import ExitStack

    with ExitStack() as ctx:
        xpool = ctx.enter_context(tc.tile_pool(name="xb", bufs=1))
        wpool = ctx.enter_context(tc.tile_pool(name="wt", bufs=1))
        pspool = ctx.enter_context(tc.tile_pool(name="ps", bufs=4, space="PSUM"))
        opool = ctx.enter_context(tc.tile_pool(name="ob", bufs=4))

        # x planes: partitions [64*im, 64*im+64) hold image im, padded.
        xb = xpool.tile([128, HP, WP], BF16)
        # weights: wt[p, t, m] = w[m, p % 64, t] (taps replicated per half)
        wt = wpool.tile([128, NTAPS, O], BF16)
        # zeroed scratch for PE warm-up (keeps HAM busy during DMA head)
        warm = wpool.tile([128, O + RPB * W], BF16)

        # warm-ups alternate quadrants exactly like the real stream (a
        # single K=128 warm LDW poisons the later (64,128)-tile pair rate
        # by ~40ns/pair — measured), keeping the whole array busy for the
        # PE_HAM activity monitor
        nc.gpsimd.memset(warm[:], 0)
        warm_ps = [
            pspool.tile([128, RPB, W], F32, tag=f"ps{im}", name=f"warm_ps{im}")
            for im in range(BPC)
        ]
        for i in range(NWARM):
            p0 = 64 * (i % 2)
            nc.tensor.matmul(
                warm_ps[i % 2][:],
                warm[p0:p0 + 64, 0:O],
                warm[p0:p0 + 64, O:O + RPB * W],
                start=True,
                stop=True,
                tile_position=(p0, 0),
            )

        # weights issue from ScalarE in parallel with band 0 on Sync --
        # serializing them on one sequencer costs ~0.7us of head latency
        nc.scalar.dma_start(out=wt[:], in_=w_ap[:])

        band_dmas = []
        for bi, (r0, n) in enumerate(_IN_BANDS):
            eng = nc.sync if bi < 2 else nc.gpsimd
            d = eng.dma_start(
                out=xb[:, r0:r0 + n, :],
                in_=xp_ap[:, r0:r0 + n, :],
            )
            if bi >= 2:
                add_dep_helper(d.ins, band_dmas[bi - 2].ins, reason="band chain")
            band_dmas.append(d)

        store_eng = {0: nc.scalar, 1: nc.sync}
        copy_eng = {0: nc.scalar.copy, 1: nc.vector.tensor_copy}
        ob_tiles = {}
        for p in range(NBLOCKS):
            r = RPB * p
            band = r // BAND
            boff = r - band * BAND
            if boff == 0:
                for im in range(BPC):
                    ob_tiles[im] = opool.tile(
                        [128, BAND, W], BF16, name=f"ob{im}_{band}", tag=f"ob{im}"
                    )
            # the very last block runs as 2+1+1-row PSUM groups so the
            # final copy+store tail is as shallow as possible
            sub_rows = [RPB] if p < NBLOCKS - 1 else [2, 1, 1]
            roff = 0
            for nrows in sub_rows:
                ps = [
                    pspool.tile([128, nrows, W], F32, tag=f"ps{im}",
                                name=f"ps{im}_{p}_{roff}")
                    for im in range(BPC)
                ]
                for t in range(NTAPS):
                    i, j = divmod(t, 3)
                    first, last = t == 0, t == NTAPS - 1
                    for im in range(BPC):
                        p0 = 64 * im
                        nc.tensor.matmul(
                            ps[im][:],
                            wt[p0:p0 + 64, t, :],
                            xb[p0:p0 + 64, r + roff + i:r + roff + i + nrows,
                               j:j + W],
                            start=first,
                            stop=last,
                            tile_position=(p0, 0),
                        )
                b0 = boff + roff
                for im in range(BPC):
                    copy_eng[im](ob_tiles[im][:, b0:b0 + nrows, :], ps[im][:])
                last_band = band == NBANDS - 1
                if last_band:
                    for im in range(BPC):
                        store_eng[im].dma_start(
                            out=out_ap[im, :, r + roff:r + roff + nrows, :],
                            in_=ob_tiles[im][:, b0:b0 + nrows, :],
                        )
                roff += nrows
            if not (band == NBANDS - 1) and boff + RPB == BAND:
                for im in range(BPC):
                    store_eng[im].dma_start(
                        out=out_ap[im, :, band * BAND:(band + 1) * BAND, :],
                        in_=ob_tiles[im][:],
                    )


def build_program():
    nc = bacc.Bacc("TRN2", target_bir_lowering=False, num_devices=N_CORES)
    x_t = nc.dram_tensor("xp", [128, HP, WP], BF16, kind="ExternalInput")
    w_t = nc.dram_tensor("wT", [128, NTAPS, O], BF16, kind="ExternalInput")
    o_t = nc.dram_tensor("out", [BPC, O, H, W], BF16, kind="ExternalOutput")
    with tile.TileContext(nc) as tc:
        _conv_body(tc, o_t.ap(), x_t.ap(), w_t.ap())
    nc.compile()
    return nc


def pack_weights(weights: np.ndarray) -> np.ndarray:
    # (O, C, 9) -> (128, 9, O) with wT[p, t, m] = weights[m, p % 64, t]
    wT = np.ascontiguousarray(np.transpose(weights, (1, 2, 0)))  # (C, 9, O)
    return np.ascontiguousarray(np.concatenate([wT, wT], axis=0)).astype(BF16NP)


def pad_input(x: np.ndarray) -> np.ndarray:
    # (B, C, H, W) -> (B, C, H+2, W+2) zero-padded bf16
    xp = np.zeros((x.shape[0], x.shape[1], HP, WP), BF16NP)
    xp[:, :, 1:1 + H, 1:1 + W] = x.astype(BF16NP)
    return xp


def run(x: np.ndarray, weights: np.ndarray, **spmd_kwargs):
    x = np.ascontiguousarray(x, dtype=np.float32)
    w = np.ascontiguousarray(weights, dtype=np.float32)
    wT = pack_weights(w)
    xp = pad_input(x)  # (B, C, HP, WP) bf16
    # per-core input: both images stacked on the channel/partition axis
    xp = xp.reshape(N_CORES, BPC * C, HP, WP)
    nc = build_program()
    in_maps = [{"xp": xp[i], "wT": wT} for i in range(N_CORES)]
    res = run_bass_kernel_spmd(nc, in_maps, list(range(N_CORES)), **spmd_kwargs)
    outs = [
        np.asarray(res.results[i]["out"]).astype(np.float32).reshape(BPC, O, H, W)
        for i in range(N_CORES)
    ]
    return np.concatenate(outs, axis=0), res


def kernel(x: np.ndarray, weights: np.ndarray) -> np.ndarray:
    out, _ = run(x, weights)
    return out


# revision 17
# speedup vs baseline: 1.0148x; 1.0148x over previous
"""Trainium2 Bass kernel for a 3x3 stride-1 pad-1 Conv2d.

Problem: x (16, 64, 112, 112) f32, weights (128, 64, 9) f32
         -> out (16, 128, 112, 112) f32  (no bias)

Strategy (8 NeuronCores, data parallel over batch):
  - Each core gets 2 images. Image 0 lives in SBUF partitions 0-63
    (64 input channels), image 1 in partitions 64-127, both stored as a
    zero-padded (114, 114) plane per channel. Padding is materialized on
    the host, so every input DMA is a contiguous fat-descriptor copy.
  - Everything is bf16 end-to-end (inputs, weights, staged outputs);
    PSUM accumulation stays fp32. bf16 halves HBM traffic and enables
    the PE's fast-weight-load path (FWL reads 2 bf16/cycle), which
    matters because LDWEIGHTS (128 cols @ 1.2 GHz) is otherwise ~45% of
    the PE-stream critical path. The host quantizes x/w to bf16 and
    upcasts the output; total rel-err ~3e-3 (gate is 2e-2).
  - Conv = 9 shift-and-matmul taps accumulated in PSUM: for each tap
    (dy, dx), matmul with lhsT = w[tap] (64 x 128: in-ch x out-ch) and
    rhs = shifted x window (64 x 448: in-ch x 4 output rows).
  - The two images' matmuls use disjoint PE row groups (rows 0-63 vs
    64-127 via tile_position) so they stream concurrently -> together
    they fill the whole 128x128 array despite the 64-deep contraction.
    Steady-state pairs run at the 448-cycle floor (~190 ns/pair).
  - Warm-up matmuls on zeroed scratch run during the DMA head so the
    PE_HAM clock gate un-throttles (1.2 -> 2.4 GHz) ASAP; the burst is
    sized to end right as the first input band lands (ending early
    risks an idle gap that slips the un-throttle by a whole HAM window).
  - Input bands are completion-chained at depth 2 (band b waits on band
    b-2) so the head band + weights get the SDMA rings to themselves --
    the rings round-robin across ACTIVE queues, so unchained later
    bands would steal head bandwidth. Bands 2+ are issued from GpSimd.
  - PSUM -> SBUF copies (f32->bf16 cast) run on ScalarE (image 0) and
    VectorE (image 1); stores are issued from ScalarE (im 0) and Sync
    (im 1) per 16-row band. The final block is split into two 2-row
    PSUM groups so its copy+store tail is halved.
"""

import numpy as np
import ml_dtypes

import concourse.bass as bass
import concourse.bacc as bacc
import concourse.mybir as mybir
import concourse.tile as tile
from concourse.bass_utils import run_bass_kernel_spmd
from concourse.tile_rust import add_dep_helper

N_CORES = 8
B, C, H, W = 16, 64, 112, 112
O = 128
BPC = B // N_CORES          # images per core
HP = H + 2                  # padded rows per image plane
WP = W + 2                  # padded cols
NTAPS = 9
RPB = 4                     # output rows per block (free dim = 4*112 = 448)
NBLOCKS = H // RPB          # 28
BAND = 16                   # output rows per output band
NBANDS = H // BAND          # 7
NWARM = 16                  # PE warm-up matmuls (8 quadrant pairs)

F32 = mybir.dt.float32
BF16 = mybir.dt.bfloat16
BF16NP = ml_dtypes.bfloat16

# input bands over padded rows: (first padded row, nrows). The head band
# covers block 0; band b>=2 is completion-chained on band b-2.
_IN_BANDS = [(0, 6), (6, 16), (22, 16), (38, 16), (54, 16), (70, 16),
             (86, 16), (102, 12)]


def _conv_body(tc, out_ap, xp_ap, w_ap):
    nc = tc.nc
    from contextlib import ExitStack

    with ExitStack() as ctx:
        xpool = ctx.enter_context(tc.tile_pool(name="xb", bufs=1))
        wpool = ctx.enter_context(tc.tile_pool(name="wt", bufs=1))
        pspool = ctx.enter_context(tc.tile_pool(name="ps", bufs=4, space="PSUM"))
        opool = ctx.enter_context(tc.tile_pool(name="ob", bufs=4))

        # x planes: partitions [64*im, 64*im+64) hold image im, padded.
        xb = xpool.tile([128, HP, WP], BF16)
        # weights: wt[p, t, m] = w[m, p % 64, t] (taps replicated per half)
        wt = wpool.tile([128, NTAPS, O], BF16)
        # zeroed scratch for PE warm-up (keeps HAM busy during DMA head)
        warm = wpool.tile([128, O + RPB * W], BF16)

        # warm-ups alternate quadrants exactly like the real stream (a
        # single K=128 warm LDW poisons the later (64,128)-tile pair rate
        # by ~40ns/pair — measured), keeping the whole array busy for the
        # PE_HAM activity monitor
        nc.gpsimd.memset(warm[:], 0)
        warm_ps = [
            pspool.tile([128, RPB, W], F32, tag=f"ps{im}", name=f"warm_ps{im}")
            for im in range(BPC)
        ]
        for i in range(NWARM):
            p0 = 64 * (i % 2)
            nc.tensor.matmul(
                warm_ps[i % 2][:],
                warm[p0:p0 + 64, 0:O],
                warm[p0:p0 + 64, O:O + RPB * W],
                start=True,
                stop=True,
                tile_position=(p0, 0),
            )

        # weights issue from ScalarE in parallel with band 0 on Sync --
        # serializing them on one sequencer costs ~0.7us of head latency
        nc.scalar.dma_start(out=wt[:], in_=w_ap[:])

        band_dmas = []
        for bi, (r0, n) in enumerate(_IN_BANDS):
            eng = nc.sync if bi < 2 else nc.gpsimd
            d = eng.dma_start(
                out=xb[:, r0:r0 + n, :],
                in_=xp_ap[:, r0:r0 + n, :],
            )
            if bi >= 2:
                add_dep_helper(d.ins, band_dmas[bi - 2].ins, reason="band chain")
            band_dmas.append(d)

        store_eng = {0: nc.scalar, 1: nc.sync}
        copy_eng = {0: nc.scalar.copy, 1: nc.vector.tensor_copy}
        ob_tiles = {}
        for p in range(NBLOCKS):
            r = RPB * p
            band = r // BAND
            boff = r - band * BAND
            if boff == 0:
                for im in range(BPC):
                    ob_tiles[im] = opool.tile(
                        [128, BAND, W], BF16, name=f"ob{im}_{band}", tag=f"ob{im}"
                    )
            # the very last block runs as two 2-row PSUM groups so the
            # final copy+store tail is half as deep
            sub_rows = [RPB] if p < NBLOCKS - 1 else [2, 2]
            roff = 0
            for nrows in sub_rows:
                ps = [
                    pspool.tile([128, nrows, W], F32, tag=f"ps{im}",
                                name=f"ps{im}_{p}_{roff}")
                    for im in range(BPC)
                ]
                for t in range(NTAPS):
                    i, j = divmod(t, 3)
                    first, last = t == 0, t == NTAPS - 1
                    for im in range(BPC):
                        p0 = 64 * im
                        nc.tensor.matmul(
                            ps[im][:],
                            wt[p0:p0 + 64, t, :],
                            xb[p0:p0 + 64, r + roff + i:r + roff + i + nrows,
                               j:j + W],
                            start=first,
                            stop=last,
                            tile_position=(p0, 0),
                        )
                b0 = boff + roff
                for im in range(BPC):
                    copy_eng[im](ob_tiles[im][:, b0:b0 + nrows, :], ps[im][:])
                last_band = band == NBANDS - 1
                if last_band:
                    for im in range(BPC):
                        store_eng[im].dma_start(
                            out=out_ap[im, :, r + roff:r + roff + nrows, :],
                            in_=ob_tiles[im][:, b0:b0 + nrows, :],
                        )
                roff += nrows
            if not (band == NBANDS - 1) and boff + RPB == BAND:
                for im in range(BPC):
                    store_eng[im].dma_start(
                        out=out_ap[im, :, band * BAND:(band + 1) * BAND, :],
                        in_=ob_tiles[im][:],
                    )


def build_program():
    nc = bacc.Bacc("TRN2", target_bir_lowering=False, num_devices=N_CORES)
    x_t = nc.dram_tensor("xp", [128, HP, WP], BF16, kind="ExternalInput")
    w_t = nc.dram_tensor("wT", [128, NTAPS, O], BF16, kind="ExternalInput")
    o_t = nc.dram_tensor("out", [BPC, O, H, W], BF16, kind="ExternalOutput")
    with tile.TileContext(nc) as tc:
        _conv_body(tc, o_t.ap(), x_t.ap(), w_t.ap())
    nc.compile()
    return nc


def pack_weights(weights: np.ndarray) -> np.ndarray:
    # (O, C, 9) -> (128, 9, O) with wT[p, t, m] = weights[m, p % 64, t]
    wT = np.ascontiguousarray(np.transpose(weights, (1, 2, 0)))  # (C, 9, O)
    return np.ascontiguousarray(np.concatenate([wT, wT], axis=0)).astype(BF16NP)


def pad_input(x: np.ndarray) -> np.ndarray:
    # (B, C, H, W) -> (B, C, H+2, W+2) zero-padded bf16
    xp = np.zeros((x.shape[0], x.shape[1], HP, WP), BF16NP)
    xp[:, :, 1:1 + H, 1:1 + W] = x.astype(BF16NP)
    return xp


def run(x: np.ndarray, weights: np.ndarray, **spmd_kwargs):
    x = np.ascontiguousarray(x, dtype=np.float32)
    w = np.ascontiguousarray(weights, dtype=np.float32)
    wT = pack_weights(w)
    xp = pad_input(x)  # (B, C, HP, WP) bf16
    # per-core input: both images stacked on the channel/partition axis
    xp = xp.reshape(N_CORES, BPC * C, HP, WP)
    nc = build_program()
    in_maps = [{"xp": xp[i], "wT": wT} for i in range(N_CORES)]
    res = run_bass_kernel_spmd(nc, in_maps, list(range(N_CORES)), **spmd_kwargs)
    outs = [
        np.asarray(res.results[i]["out"]).astype(np.float32).reshape(BPC, O, H, W)
        for i in range(N_CORES)
    ]
    return np.concatenate(outs, axis=0), res


def kernel(x: np.ndarray, weights: np.ndarray) -> np.ndarray:
    out, _ = run(x, weights)
    return out


# revision 18
# speedup vs baseline: 1.0157x; 1.0009x over previous
"""Trainium2 Bass kernel for a 3x3 stride-1 pad-1 Conv2d.

Problem: x (16, 64, 112, 112) f32, weights (128, 64, 9) f32
         -> out (16, 128, 112, 112) f32  (no bias)

Strategy (8 NeuronCores, data parallel over batch):
  - Each core gets 2 images. Image 0 lives in SBUF partitions 0-63
    (64 input channels), image 1 in partitions 64-127, both stored as a
    zero-padded (114, 114) plane per channel. Padding is materialized on
    the host, so every input DMA is a contiguous fat-descriptor copy.
  - Everything is bf16 end-to-end (inputs, weights, staged outputs);
    PSUM accumulation stays fp32. bf16 halves HBM traffic and enables
    the PE's fast-weight-load path (FWL reads 2 bf16/cycle), which
    matters because LDWEIGHTS (128 cols @ 1.2 GHz) is otherwise ~45% of
    the PE-stream critical path. The host quantizes x/w to bf16 and
    upcasts the output; total rel-err ~3e-3 (gate is 2e-2).
  - Conv = 9 shift-and-matmul taps accumulated in PSUM: for each tap
    (dy, dx), matmul with lhsT = w[tap] (64 x 128: in-ch x out-ch) and
    rhs = shifted x window (64 x 448: in-ch x 4 output rows).
  - The two images' matmuls use disjoint PE row groups (rows 0-63 vs
    64-127 via tile_position) so they stream concurrently -> together
    they fill the whole 128x128 array despite the 64-deep contraction.
    Steady-state pairs run at the 448-cycle floor (~190 ns/pair).
  - Warm-up matmuls on zeroed scratch run during the DMA head so the
    PE_HAM clock gate un-throttles (1.2 -> 2.4 GHz) ASAP; the burst is
    sized to end right as the first input band lands (ending early
    risks an idle gap that slips the un-throttle by a whole HAM window).
  - Input bands are completion-chained at depth 2 (band b waits on band
    b-2) so the head band + weights get the SDMA rings to themselves --
    the rings round-robin across ACTIVE queues, so unchained later
    bands would steal head bandwidth. Bands 2+ are issued from GpSimd.
  - PSUM -> SBUF copies (f32->bf16 cast) run on ScalarE (image 0) and
    VectorE (image 1); stores are issued from ScalarE (im 0) and Sync
    (im 1) per 16-row band. The final block is split into two 2-row
    PSUM groups so its copy+store tail is halved.
"""

import numpy as np
import ml_dtypes

import concourse.bass as bass
import concourse.bacc as bacc
import concourse.mybir as mybir
import concourse.tile as tile
from concourse.bass_utils import run_bass_kernel_spmd
from concourse.tile_rust import add_dep_helper

N_CORES = 8
B, C, H, W = 16, 64, 112, 112
O = 128
BPC = B // N_CORES          # images per core
HP = H + 2                  # padded rows per image plane
WP = W + 2                  # padded cols
NTAPS = 9
RPB = 4                     # output rows per block (free dim = 4*112 = 448)
NBLOCKS = H // RPB          # 28
BAND = 16                   # output rows per output band
NBANDS = H // BAND          # 7
NWARM = 16                  # PE warm-up matmuls (8 quadrant pairs)

F32 = mybir.dt.float32
BF16 = mybir.dt.bfloat16
BF16NP = ml_dtypes.bfloat16

# input bands over padded rows: (first padded row, nrows). The head band
# covers block 0; band b>=2 is completion-chained on band b-2.
_IN_BANDS = [(0, 6), (6, 16), (22, 16), (38, 16), (54, 16), (70, 16),
             (86, 16), (102, 12)]


def _conv_body(tc, out_ap, xp_ap, w_ap):
    nc = tc.nc
    from contextlib import ExitStack

    with ExitStack() as ctx:
        xpool = ctx.enter_context(tc.tile_pool(name="xb", bufs=1))
        wpool = ctx.enter_context(tc.tile_pool(name="wt", bufs=1))
        pspool = ctx.enter_context(tc.tile_pool(name="ps", bufs=4, space="PSUM"))
        opool = ctx.enter_context(tc.tile_pool(name="ob", bufs=4))

        # x planes: partitions [64*im, 64*im+64) hold image im, padded.
        xb = xpool.tile([128, HP, WP], BF16)
        # weights: wt[p, t, m] = w[m, p % 64, t] (taps replicated per half)
        wt = wpool.tile([128, NTAPS, O], BF16)
        # zeroed scratch for PE warm-up (keeps HAM busy during DMA head)
        warm = wpool.tile([128, O + RPB * W], BF16)

        # warm-ups alternate quadrants exactly like the real stream (a
        # single K=128 warm LDW poisons the later (64,128)-tile pair rate
        # by ~40ns/pair — measured), keeping the whole array busy for the
        # PE_HAM activity monitor
        nc.gpsimd.memset(warm[:], 0)
        warm_ps = [
            pspool.tile([128, RPB, W], F32, tag=f"ps{im}", name=f"warm_ps{im}")
            for im in range(BPC)
        ]
        for i in range(NWARM):
            p0 = 64 * (i % 2)
            nc.tensor.matmul(
                warm_ps[i % 2][:],
                warm[p0:p0 + 64, 0:O],
                warm[p0:p0 + 64, O:O + RPB * W],
                start=True,
                stop=True,
                tile_position=(p0, 0),
            )

        # weights issue from ScalarE in parallel with band 0 on Sync --
        # serializing them on one sequencer costs ~0.7us of head latency
        nc.scalar.dma_start(out=wt[:], in_=w_ap[:])

        # all bands issue from Sync: chaining semantics are the same as
        # issuing from GpSimd, but GpSimd's SWDGE path costs 8 extra
        # semaphores, each adding ~115ns to the runtime's exit sweep
        band_dmas = []
        for bi, (r0, n) in enumerate(_IN_BANDS):
            d = nc.sync.dma_start(
                out=xb[:, r0:r0 + n, :],
                in_=xp_ap[:, r0:r0 + n, :],
            )
            if bi >= 2:
                add_dep_helper(d.ins, band_dmas[bi - 2].ins, reason="band chain")
            band_dmas.append(d)

        store_eng = {0: nc.scalar, 1: nc.sync}
        copy_eng = {0: nc.scalar.copy, 1: nc.vector.tensor_copy}
        ob_tiles = {}
        for p in range(NBLOCKS):
            r = RPB * p
            band = r // BAND
            boff = r - band * BAND
            if boff == 0:
                for im in range(BPC):
                    ob_tiles[im] = opool.tile(
                        [128, BAND, W], BF16, name=f"ob{im}_{band}", tag=f"ob{im}"
                    )
            # the very last block runs as two 2-row PSUM groups so the
            # final copy+store tail is half as deep
            sub_rows = [RPB] if p < NBLOCKS - 1 else [2, 2]
            roff = 0
            for nrows in sub_rows:
                ps = [
                    pspool.tile([128, nrows, W], F32, tag=f"ps{im}",
                                name=f"ps{im}_{p}_{roff}")
                    for im in range(BPC)
                ]
                for t in range(NTAPS):
                    i, j = divmod(t, 3)
                    first, last = t == 0, t == NTAPS - 1
                    for im in range(BPC):
                        p0 = 64 * im
                        nc.tensor.matmul(
                            ps[im][:],
                            wt[p0:p0 + 64, t, :],
                            xb[p0:p0 + 64, r + roff + i:r + roff + i + nrows,
                               j:j + W],
                            start=first,
                            stop=last,
                            tile_position=(p0, 0),
                        )
                b0 = boff + roff
                for im in range(BPC):
                    copy_eng[im](ob_tiles[im][:, b0:b0 + nrows, :], ps[im][:])
                last_band = band == NBANDS - 1
                if last_band:
                    for im in range(BPC):
                        store_eng[im].dma_start(
                            out=out_ap[im, :, r + roff:r + roff + nrows, :],
                            in_=ob_tiles[im][:, b0:b0 + nrows, :],
                        )
                roff += nrows
            if not (band == NBANDS - 1) and boff + RPB == BAND:
                for im in range(BPC):
                    store_eng[im].dma_start(
                        out=out_ap[im, :, band * BAND:(band + 1) * BAND, :],
                        in_=ob_tiles[im][:],
                    )


def build_program():
    nc = bacc.Bacc("TRN2", target_bir_lowering=False, num_devices=N_CORES)
    x_t = nc.dram_tensor("xp", [128, HP, WP], BF16, kind="ExternalInput")
    w_t = nc.dram_tensor("wT", [128, NTAPS, O], BF16, kind="ExternalInput")
    o_t = nc.dram_tensor("out", [BPC, O, H, W], BF16, kind="ExternalOutput")
    with tile.TileContext(nc) as tc:
        _conv_body(tc, o_t.ap(), x_t.ap(), w_t.ap())
    nc.compile()
    return nc


def pack_weights(weights: np.ndarray) -> np.ndarray:
    # (O, C, 9) -> (128, 9, O) with wT[p, t, m] = weights[m, p % 64, t]
    wT = np.ascontiguousarray(np.transpose(weights, (1, 2, 0)))  # (C, 9, O)
    return np.ascontiguousarray(np.concatenate([wT, wT], axis=0)).astype(BF16NP)


def pad_input(x: np.ndarray) -> np.ndarray:
    # (B, C, H, W) -> (B, C, H+2, W+2) zero-padded bf16
    xp = np.zeros((x.shape[0], x.shape[1], HP, WP), BF16NP)
    xp[:, :, 1:1 + H, 1:1 + W] = x.astype(BF16NP)
    return xp


def run(x: np.ndarray, weights: np.ndarray, **spmd_kwargs):
    x = np.ascontiguousarray(x, dtype=np.float32)
    w = np.ascontiguousarray(weights, dtype=np.float32)
    wT = pack_weights(w)
    xp = pad_input(x)  # (B, C, HP, WP) bf16
    # per-core input: both images stacked on the channel/partition axis
    xp = xp.reshape(N_CORES, BPC * C, HP, WP)
    nc = build_program()
    in_maps = [{"xp": xp[i], "wT": wT} for i in range(N_CORES)]
    res = run_bass_kernel_spmd(nc, in_maps, list(range(N_CORES)), **spmd_kwargs)
    outs = [
        np.asarray(res.results[i]["out"]).astype(np.float32).reshape(BPC, O, H, W)
        for i in range(N_CORES)
    ]
    return np.concatenate(outs, axis=0), res


def kernel(x: np.ndarray, weights: np.ndarray) -> np.ndarray:
    out, _ = run(x, weights)
    return out


# revision 19
# speedup vs baseline: 1.0329x; 1.0170x over previous
"""Trainium2 Bass kernel for a 3x3 stride-1 pad-1 Conv2d.

Problem: x (16, 64, 112, 112) f32, weights (128, 64, 9) f32
         -> out (16, 128, 112, 112) f32  (no bias)

Strategy (8 NeuronCores, data parallel over batch):
  - Each core gets 2 images. Image 0 lives in SBUF partitions 0-63
    (64 input channels), image 1 in partitions 64-127, both stored as a
    zero-padded (114, 114) plane per channel. Padding is materialized on
    the host, so every input DMA is a contiguous fat-descriptor copy.
  - Everything is bf16 end-to-end (inputs, weights, staged outputs);
    PSUM accumulation stays fp32. bf16 halves HBM traffic and enables
    the PE's fast-weight-load path (FWL reads 2 bf16/cycle), which
    matters because LDWEIGHTS (128 cols @ 1.2 GHz) is otherwise ~45% of
    the PE-stream critical path. The host quantizes x/w to bf16 and
    upcasts the output; total rel-err ~3e-3 (gate is 2e-2).
  - Conv = 9 shift-and-matmul taps accumulated in PSUM: for each tap
    (dy, dx), matmul with lhsT = w[tap] (64 x 128: in-ch x out-ch) and
    rhs = shifted x window (64 x 448: in-ch x 4 output rows).
  - The two images' matmuls use disjoint PE row groups (rows 0-63 vs
    64-127 via tile_position) so they stream concurrently -> together
    they fill the whole 128x128 array despite the 64-deep contraction.
    Steady-state pairs run at the 448-cycle floor (~190 ns/pair).
  - Warm-up matmuls on zeroed scratch run during the DMA head so the
    PE_HAM clock gate un-throttles (1.2 -> 2.4 GHz) ASAP; the burst is
    sized to end right as the first input band lands (ending early
    risks an idle gap that slips the un-throttle by a whole HAM window).
  - Input bands are completion-chained at depth 2 (band b waits on band
    b-2) so the head band + weights get the SDMA rings to themselves --
    the rings round-robin across ACTIVE queues, so unchained later
    bands would steal head bandwidth. Bands 2+ are issued from GpSimd.
  - PSUM -> SBUF copies (f32->bf16 cast) run on ScalarE (image 0) and
    VectorE (image 1); stores are issued from ScalarE (im 0) and Sync
    (im 1) per 16-row band. The final block is split into two 2-row
    PSUM groups so its copy+store tail is halved.
"""

import numpy as np
import ml_dtypes

import concourse.bass as bass
import concourse.bacc as bacc
import concourse.mybir as mybir
import concourse.tile as tile
from concourse.bass_utils import run_bass_kernel_spmd
from concourse.tile_rust import add_dep_helper

N_CORES = 8
B, C, H, W = 16, 64, 112, 112
O = 128
BPC = B // N_CORES          # images per core
HP = H + 2                  # padded rows per image plane
WP = W + 2                  # padded cols
NTAPS = 9
RPB = 4                     # output rows per block (free dim = 4*112 = 448)
NBLOCKS = H // RPB          # 28
BAND = 16                   # output rows per output band
NBANDS = H // BAND          # 7
NWARM = 16                  # PE warm-up matmuls (8 quadrant pairs)

F32 = mybir.dt.float32
BF16 = mybir.dt.bfloat16
BF16NP = ml_dtypes.bfloat16

# input bands over padded rows: (first padded row, nrows). The head band
# covers block 0; band b>=2 is completion-chained on band b-2.
_IN_BANDS = [(0, 6), (6, 16), (22, 16), (38, 16), (54, 16), (70, 16),
             (86, 16), (102, 12)]


def _conv_body(tc, out_ap, xp_ap, w_ap):
    nc = tc.nc
    from contextlib import ExitStack

    with ExitStack() as ctx:
        xpool = ctx.enter_context(tc.tile_pool(name="xb", bufs=1))
        wpool = ctx.enter_context(tc.tile_pool(name="wt", bufs=1))
        pspool = ctx.enter_context(tc.tile_pool(name="ps", bufs=4, space="PSUM"))
        opool = ctx.enter_context(tc.tile_pool(name="ob", bufs=4))

        # x planes: partitions [64*im, 64*im+64) hold image im, padded.
        xb = xpool.tile([128, HP, WP], BF16)
        # weights: wt[p, t, m] = w[m, p % 64, t] (taps replicated per half)
        wt = wpool.tile([128, NTAPS, O], BF16)
        # zeroed scratch for PE warm-up (keeps HAM busy during DMA head)
        warm = wpool.tile([128, O + RPB * W], BF16)

        # warm-ups alternate quadrants exactly like the real stream (a
        # single K=128 warm LDW poisons the later (64,128)-tile pair rate
        # by ~40ns/pair — measured), keeping the whole array busy for the
        # PE_HAM activity monitor
        nc.gpsimd.memset(warm[:], 0)
        warm_ps = [
            pspool.tile([128, RPB, W], F32, tag=f"ps{im}", name=f"warm_ps{im}")
            for im in range(BPC)
        ]
        for i in range(NWARM):
            p0 = 64 * (i % 2)
            nc.tensor.matmul(
                warm_ps[i % 2][:],
                warm[p0:p0 + 64, 0:O],
                warm[p0:p0 + 64, O:O + RPB * W],
                start=True,
                stop=True,
                tile_position=(p0, 0),
            )

        # weights issue from ScalarE in parallel with band 0 on Sync --
        # serializing them on one sequencer costs ~0.7us of head latency
        nc.scalar.dma_start(out=wt[:], in_=w_ap[:])

        # all bands issue from Sync: chaining semantics are the same as
        # issuing from GpSimd, but GpSimd's SWDGE path costs 8 extra
        # semaphores, each adding ~115ns to the runtime's exit sweep
        band_dmas = []
        for bi, (r0, n) in enumerate(_IN_BANDS):
            d = nc.sync.dma_start(
                out=xb[:, r0:r0 + n, :],
                in_=xp_ap[:, r0:r0 + n, :],
            )
            if bi >= 2:
                add_dep_helper(d.ins, band_dmas[bi - 2].ins, reason="band chain")
            band_dmas.append(d)

        store_eng = {0: nc.scalar, 1: nc.sync}
        copy_eng = {0: nc.scalar.copy, 1: nc.vector.tensor_copy}
        ob_tiles = {}
        for p in range(NBLOCKS):
            r = RPB * p
            band = r // BAND
            boff = r - band * BAND
            if boff == 0:
                for im in range(BPC):
                    ob_tiles[im] = opool.tile(
                        [128, BAND, W], BF16, name=f"ob{im}_{band}", tag=f"ob{im}"
                    )
            # the very last block runs as two 2-row PSUM groups so the
            # final copy+store tail is half as deep
            sub_rows = [RPB] if p < NBLOCKS - 1 else [2, 2]
            roff = 0
            for nrows in sub_rows:
                ps = [
                    pspool.tile([128, nrows, W], F32, tag=f"ps{im}",
                                name=f"ps{im}_{p}_{roff}")
                    for im in range(BPC)
                ]
                for t in range(NTAPS):
                    i, j = divmod(t, 3)
                    first, last = t == 0, t == NTAPS - 1
                    for im in range(BPC):
                        p0 = 64 * im
                        nc.tensor.matmul(
                            ps[im][:],
                            wt[p0:p0 + 64, t, :],
                            xb[p0:p0 + 64, r + roff + i:r + roff + i + nrows,
                               j:j + W],
                            start=first,
                            stop=last,
                            tile_position=(p0, 0),
                        )
                b0 = boff + roff
                for im in range(BPC):
                    copy_eng[im](ob_tiles[im][:, b0:b0 + nrows, :], ps[im][:])
                last_band = band == NBANDS - 1
                if last_band:
                    for im in range(BPC):
                        store_eng[im].dma_start(
                            out=out_ap[im, :, r + roff:r + roff + nrows, :],
                            in_=ob_tiles[im][:, b0:b0 + nrows, :],
                        )
                roff += nrows
            if not (band == NBANDS - 1) and boff + RPB == BAND:
                for im in range(BPC):
                    store_eng[im].dma_start(
                        out=out_ap[im, :, band * BAND:(band + 1) * BAND, :],
                        in_=ob_tiles[im][:],
                    )


def build_program():
    nc = bacc.Bacc("TRN2", target_bir_lowering=False, num_devices=N_CORES)
    x_t = nc.dram_tensor("xp", [128, HP, WP], BF16, kind="ExternalInput")
    w_t = nc.dram_tensor("wT", [128, NTAPS, O], BF16, kind="ExternalInput")
    o_t = nc.dram_tensor("out", [BPC, O, H, W], BF16, kind="ExternalOutput")
    with tile.TileContext(nc) as tc:
        _conv_body(tc, o_t.ap(), x_t.ap(), w_t.ap())
    nc.compile()
    return nc


def pack_weights(weights: np.ndarray) -> np.ndarray:
    # (O, C, 9) -> (128, 9, O) with wT[p, t, m] = weights[m, p % 64, t]
    wT = np.ascontiguousarray(np.transpose(weights, (1, 2, 0)))  # (C, 9, O)
    return np.ascontiguousarray(np.concatenate([wT, wT], axis=0)).astype(BF16NP)


def pad_input(x: np.ndarray) -> np.ndarray:
    # (B, C, H, W) -> (B, C, H+2, W+2) zero-padded bf16
    xp = np.zeros((x.shape[0], x.shape[1], HP, WP), BF16NP)
    xp[:, :, 1:1 + H, 1:1 + W] = x.astype(BF16NP)
    return xp


def run(x: np.ndarray, weights: np.ndarray, **spmd_kwargs):
    x = np.ascontiguousarray(x, dtype=np.float32)
    w = np.ascontiguousarray(weights, dtype=np.float32)
    wT = pack_weights(w)
    xp = pad_input(x)  # (B, C, HP, WP) bf16
    # per-core input: both images stacked on the channel/partition axis
    xp = xp.reshape(N_CORES, BPC * C, HP, WP)
    in_maps = [{"xp": xp[i], "wT": wT} for i in range(N_CORES)]
    # the neuron runtime occasionally throws transient INTERNAL errors
    # under load; a fresh build+run retry has always succeeded
    for attempt in range(3):
        try:
            nc = build_program()
            res = run_bass_kernel_spmd(nc, in_maps, list(range(N_CORES)),
                                       **spmd_kwargs)
            break
        except Exception:
            if attempt == 2:
                raise
    outs = [
        np.asarray(res.results[i]["out"]).astype(np.float32).reshape(BPC, O, H, W)
        for i in range(N_CORES)
    ]
    return np.concatenate(outs, axis=0), res


def kernel(x: np.ndarray, weights: np.ndarray) -> np.ndarray:
    out, _ = run(x, weights)
    return out
